# revision 15
# baseline (speedup 1.0000x reference)
"""Cross-attention kernel for Trainium2, SPMD across 8 NeuronCores.

Problem: out = softmax(mask(enc_q@Wq^T @ (enc_k@Wk^T)^T / sqrt(D))) @ (x@Wv^T)
Shapes: B=4, SQ=SKV=2048, D=2048, fp32 inputs.

Sharding: q-row parallel. Each core owns 1024 q rows (half a batch element).
The chain is reassociated so every stage is row-parallel in q with zero
cross-core communication. The two leading projections are merged by folding
the weight-weight product on the host:
    Wqk  = (Wq^T/sqrt(D)) @ Wk                        (host, one D^3 sgemm)
    q2^T = Wqk-stationary matmul over enc_q^T         [d, m]
    s^T  = enc_k^T-stationary matmul -> scores transposed [s, m]
    u^T  = exp(s^T) * keep^T                          (masked, unnormalized)
    Z    = ones^T @ u^T                               (softmax denominators)
    ax^T = x-stationary matmul over u^T               [d, m]
    out  = (ax^T-stationary @ Wv^T) * (1/Z)           [m, e]
All matmuls run in bf16 with fp32 PSUM accumulation; the final output is
normalized and stored in fp32. The softmax max-subtraction is skipped:
scores are ~N(0,1) after the 1/sqrt(D) scale (|s| <= ||q||*||k|| ~ 45 in the
worst case, far below fp32/bf16 exp overflow), and masked entries are exact
zeros via the multiplicative 0/1 mask.
"""

import os
import sys

import numpy as np

for _p in ("/opt/trn_rl_repo",):
    if _p not in sys.path and os.path.isdir(_p):
        sys.path.append(_p)

import ml_dtypes

import concourse.bass as bass
import concourse.mybir as mybir
import concourse.tile as tile
from concourse.bass_utils import run_bass_kernel_spmd
from concourse.vector_clock import ScopedClock

BF16 = mybir.dt.bfloat16
F32 = mybir.dt.float32
NP_BF16 = ml_dtypes.bfloat16

P = 128          # partitions
D = 2048         # model dim
S = 2048         # kv sequence per batch
M = 1024         # q rows per core
B = 4
N_CORES = 8
DT = D // P      # 16 d-tiles
ST = S // P      # 16 s-tiles
ET = D // P      # 16 e-tiles
MT = M // P      # 8 m-tiles
MC = M // 512    # 2 moving chunks of 512
EC = D // 512    # 4 output e-chunks of 512
EXP = mybir.ActivationFunctionType.Exp


# The walrus build in this image lowers at most ONE sync wait per
# instruction (any class) — more fails codegen with "Too many sync wait
# commands".
_MAX_WAITS = 1


class _SplitDrainTileContext(tile.TileContext):
    """TileContext that legalizes per-instruction semaphore waits for the
    walrus build in this image, which lowers at most ~2 sync waits per
    instruction ("Too many sync wait commands" in codegen otherwise).

    - Engine instructions with >2 waits get a chain of same-engine NOPs
      inserted immediately before them, each carrying <=2 of the waits
      (engine streams are strictly in-order, so this is equivalent).
    - DMA instructions execute on autonomous HWDGE queue procs, where a
      preceding engine NOP would not gate them. Excess waits move to an
      SP NOP chain that increments a fresh semaphore; the DMA keeps one
      original wait plus a wait on that semaphore.
    - The exit drain's all-proc wait set is split one-per-NOP the same way.
    Fix semaphores are cleared in the tail alongside Tile's own."""

    def _legalize_waits(self):
        nc = self.nc
        try:
            from concourse.tile_sem_assignment import DMAInst as _DMAInst
        except Exception:
            _DMAInst = ()
        # One shared forwarding semaphore for all DMA fixes: fix chain k
        # (SP, in-order) increments it to k, and the k-th fixed DMA waits
        # for >= k. SP in-order execution makes the value monotonic and
        # each chain's conditions transitively satisfied.
        fix_sem = None
        fix_val = 0
        n_fix = 0
        for f in nc.m.functions:
            for blk in f.blocks:
                il = blk.instructions
                out_l = []
                for inst in il:
                    si = getattr(inst, "sync_info", None)
                    waits = list(si.on_wait) if (si and si.on_wait) else []
                    if len(waits) <= _MAX_WAITS:
                        out_l.append(inst)
                        continue
                    is_dma = isinstance(inst, _DMAInst) or "DMA" in type(inst).__name__
                    if is_dma:
                        # all waits -> SP NOP chain (1 wait each) -> shared sem
                        if fix_sem is None:
                            fix_sem = nc.alloc_semaphore("waitfix_sem")
                        fix_val += 1
                        for ci, w in enumerate(waits):
                            upd = []
                            if ci == len(waits) - 1:
                                upd = [mybir.SyncUpdate(
                                    sync_type="semaphore", id=fix_sem.num,
                                    ant_name=fix_sem.name, update_mode="sem-inc",
                                    update_value=1,
                                )]
                            out_l.append(mybir.InstNoOp(
                                name=f"I-waitfix{n_fix}-{ci}",
                                sync_info=mybir.SyncInfo(on_wait=[w], on_update=upd),
                                bass_nofuse=True,
                                engine=mybir.EngineType.SP,
                            ))
                        si.on_wait = [mybir.SyncWait(
                            sync_type="semaphore", id=fix_sem.num,
                            ant_name=fix_sem.name,
                            wait_mode="sem-ge-imm", wait_value=fix_val,
                        )]
                    else:
                        # same-engine NOP chain before the instruction,
                        # one wait per NOP (engine streams are in-order)
                        keep = waits[-_MAX_WAITS:]
                        excess = waits[:-_MAX_WAITS]
                        for ci, w in enumerate(excess):
                            out_l.append(mybir.InstNoOp(
                                name=f"I-waitfix{n_fix}-{ci}",
                                sync_info=mybir.SyncInfo(on_wait=[w], on_update=[]),
                                bass_nofuse=True,
                                engine=inst.engine,
                            ))
                        si.on_wait = keep
                    n_fix += 1
                    out_l.append(inst)
                if len(out_l) != len(il):
                    il[:] = out_l
        return fix_sem

    def _drain_and_barrier(self, tick_clock, wait_clock):
        nc = self.nc
        fix_sem = self._legalize_waits()
        fix_sems = [fix_sem] if fix_sem is not None else []
        probe = nc.sync.nop(hint="drain_wait_probe")
        wait_clock.add_sem_waits(
            probe.ins, ScopedClock({None: tick_clock.global_clock})
        )
        waits = list(probe.ins.sync_info.on_wait)
        probe.ins.sync_info.on_wait = waits[:1]
        for w in waits[1:]:
            nop = nc.sync.nop(hint="drain_wait_split")
            if nop.ins.sync_info is None:
                nop.ins.sync_info = mybir.SyncInfo(on_wait=[w], on_update=[])
            else:
                nop.ins.sync_info.on_wait = [w]
        nc.sync.drain()
        nc.all_engine_barrier()
        assert self.sems is not None
        popped = nc._tile_sem_poison_stack.pop()
        assert popped is self._sem_poison
        nc.clear_and_free_semaphores(
            list(self.sems.allocated().values()) + fix_sems
        )
        nc.all_engine_barrier()


def build_program() -> bass.Bass:
    nc = bass.Bass("TRN2", target_bir_lowering=False, debug=False, num_devices=1)

    eqT = nc.dram_tensor("eqT", [DT // 4, P, 4, M], BF16, kind="ExternalInput").ap()
    wqk1 = nc.dram_tensor("wqk1", [DT, P, ET, P], BF16, kind="ExternalInput").ap()
    ek1 = nc.dram_tensor("ek1", [ST, P, DT, P], BF16, kind="ExternalInput").ap()
    keepT = nc.dram_tensor("keepT", [ST, P, M], BF16, kind="ExternalInput").ap()
    x1 = nc.dram_tensor("x1", [DT, P, ST, P], BF16, kind="ExternalInput").ap()
    wv1 = nc.dram_tensor("wv1", [EC, P, DT, 512], BF16, kind="ExternalInput").ap()
    out = nc.dram_tensor("out", [M, D], F32, kind="ExternalOutput").ap()

    with _SplitDrainTileContext(nc) as tc:
        with (
            tc.tile_pool(name="acts", bufs=1) as acts,
            tc.tile_pool(name="wstream", bufs=4) as wstream,
            tc.tile_pool(name="wvstream", bufs=2) as wvstream,
            tc.tile_pool(name="keeps", bufs=2) as keeps,
            tc.tile_pool(name="small", bufs=1) as small,
            tc.tile_pool(name="etmps", bufs=4) as etmps,
            tc.tile_pool(name="outs", bufs=4) as outs,
            tc.tile_pool(name="ps", bufs=6, space="PSUM") as ps,
            tc.tile_pool(name="psz", bufs=2, space="PSUM") as psz,
            tc.tile_pool(name="dram", bufs=1, space="DRAM") as dram,
        ):
            # ---- PE pre-warm: dummy matmuls with no input deps keep the
            # HAM busy-window filled during the input DMA prefill (~10.8us:
            # HWDGE queues take ~6us to start streaming, then w0+eqT must
            # land), so the first real matmuls run at K=8/8 (2.4 GHz) with
            # their operands resident. 40 N=512 matmuls span ~10.3us
            # (8 cold @427ns + 32 warm @216ns).
            warm_src = small.tile([P, 512], BF16, tag="warmsrc")
            nc.vector.memset(warm_src[:], 0.0)
            warm_ps = ps.tile([P, 512], F32, tag="mm", name="warm")
            for _ in range(40):
                nc.tensor.matmul(
                    warm_ps[:], warm_src[:, :P], warm_src[:],
                    start=True, stop=True,
                )

            # ---- resident activations (two rotating slots per tag) ----
            # DMA trigger instructions serialize on the SP engine (~650ns
            # each), so issue order = availability order. The first stage-A
            # matmul needs wqk chunk 0 + eqT chunk 0: put those first.
            # eqT is split per-dt so matmul et can start as chunk et lands.
            w0_t = wstream.tile([P, ET, P], BF16, tag="wchunk", name="w0")
            nc.sync.dma_start(w0_t[:], wqk1[0])
            # eqT arrives in 4 batched DMAs of 4 d-tiles each (fewer ~640ns
            # SP trigger slots -> the wqk stream's triggers issue sooner).
            EQG = 4
            eqT_t = [
                acts.tile([P, EQG, M], BF16, tag="eqt", bufs=DT // EQG,
                          name=f"eqT{g}")
                for g in range(DT // EQG)
            ]
            for g in range(DT // EQG):
                nc.sync.dma_start(eqT_t[g][:], eqT[g])

            # ---- A: q2T[d, m] = (Wqk stationary) over enc_q^T ----
            # Wqk = (Wq^T/sqrt(D)) @ Wk is folded on the host, merging the
            # q- and k-projections into one device stage.
            q2T_sb = acts.tile([P, DT, M], BF16, tag="actA", bufs=1)
            for db in range(DT):
                if db == 0:
                    w_t = w0_t
                else:
                    w_t = wstream.tile([P, ET, P], BF16, tag="wchunk")
                    nc.sync.dma_start(w_t[:], wqk1[db])
                psums = [ps.tile([P, 512], F32, tag="mm", name=f"mm{i}") for i in range(MC)]
                for et in range(ET):
                    for mc in range(MC):
                        nc.tensor.matmul(
                            psums[mc][:],
                            w_t[:, et, :],
                            eqT_t[et // EQG][:, et % EQG, mc * 512:(mc + 1) * 512],
                            start=(et == 0),
                            stop=(et == ET - 1),
                        )
                for mc in range(MC):
                    nc.scalar.copy(q2T_sb[:, db, mc * 512:(mc + 1) * 512], psums[mc][:])

            # ---- S3: uT[s, m] = exp(scoresT) * keepT ----
            uT_sb = acts.tile([P, ST, M], BF16, tag="actB", bufs=1)
            for sb in range(ST):
                w_t = wstream.tile([P, DT, P], BF16, tag="wchunk")
                nc.sync.dma_start(w_t[:], ek1[sb])
                k_t = keeps.tile([P, M], BF16, tag="keep")
                nc.sync.dma_start(k_t[:], keepT[sb])
                psums = [ps.tile([P, 512], F32, tag="mm", name=f"mm{i}") for i in range(MC)]
                for dt in range(DT):
                    for mc in range(MC):
                        nc.tensor.matmul(
                            psums[mc][:],
                            w_t[:, dt, :],
                            q2T_sb[:, dt, mc * 512:(mc + 1) * 512],
                            start=(dt == 0),
                            stop=(dt == DT - 1),
                        )
                for mc in range(MC):
                    e_t = etmps.tile([P, 512], BF16, tag="etmp")
                    nc.scalar.activation(e_t[:], psums[mc][:], EXP)
                    nc.vector.tensor_mul(
                        out=uT_sb[:, sb, mc * 512:(mc + 1) * 512],
                        in0=e_t[:],
                        in1=k_t[:, mc * 512:(mc + 1) * 512],
                    )

            # ---- Z: softmax denominators, then r = 1/Z in [m-partition] form ----
            # 8-wide all-ones stationary (1-col stationary matmuls don't
            # pipeline: +~180ns each) and mc-alternating PSUM banks so
            # consecutive Z matmuls ping-pong banks like the main stages.
            ones_sb = small.tile([P, 8], BF16, tag="ones")
            nc.vector.memset(ones_sb[:], 1.0)
            r_row = small.tile([1, M], F32, tag="rrow")
            zps = [psz.tile([8, 512], F32, tag="z", name=f"z{i}") for i in range(MC)]
            for sb in range(ST):
                for mc in range(MC):
                    nc.tensor.matmul(
                        zps[mc][:],
                        ones_sb[:],
                        uT_sb[:, sb, mc * 512:(mc + 1) * 512],
                        start=(sb == 0),
                        stop=(sb == ST - 1),
                    )
            for mc in range(MC):
                nc.vector.reciprocal(r_row[:, mc * 512:(mc + 1) * 512], zps[mc][0:1, :])
            r_dram = dram.tile([M], F32)
            nc.sync.dma_start(r_dram[None, :], r_row[:])
            r_pt = small.tile([P, MT], F32, tag="rpt")
            nc.sync.dma_start(r_pt[:], r_dram.rearrange("(t p) -> p t", p=P))

            # ---- S5: axT[d, m] = (x stationary) over uT ----
            axT_sb = acts.tile([P, DT, M], BF16, tag="actA", bufs=1)
            for db in range(DT):
                w_t = wstream.tile([P, ST, P], BF16, tag="wchunk")
                nc.sync.dma_start(w_t[:], x1[db])
                psums = [ps.tile([P, 512], F32, tag="mm", name=f"mm{i}") for i in range(MC)]
                for st in range(ST):
                    for mc in range(MC):
                        nc.tensor.matmul(
                            psums[mc][:],
                            w_t[:, st, :],
                            uT_sb[:, st, mc * 512:(mc + 1) * 512],
                            start=(st == 0),
                            stop=(st == ST - 1),
                        )
                for mc in range(MC):
                    nc.scalar.copy(axT_sb[:, db, mc * 512:(mc + 1) * 512], psums[mc][:])

            # ---- S6: out[m, e] = (axT stationary @ Wv^T) * r ----
            # The very last tile runs as two 256-wide halves so its r-mul +
            # store-DMA overlap the second half's matmuls, shortening the
            # exposed tail after the final matmul.
            for ec in range(EC):
                wv_t = wvstream.tile([P, DT, 512], BF16, tag="wvchunk")
                nc.sync.dma_start(wv_t[:], wv1[ec])
                for mt in range(MT):
                    last = (ec == EC - 1) and (mt == MT - 1)
                    po = ps.tile([P, 512], F32, tag="mm", name="mmo")
                    halves = ((0, 256), (256, 512)) if last else ((0, 512),)
                    for lo, hi in halves:
                        for dt in range(DT):
                            nc.tensor.matmul(
                                po[:, lo:hi],
                                axT_sb[:, dt, mt * P:(mt + 1) * P],
                                wv_t[:, dt, lo:hi],
                                start=(dt == 0),
                                stop=(dt == DT - 1),
                            )
                        o_t = outs.tile([P, hi - lo], F32,
                                        tag="ostage" if not last else "ostage2")
                        nc.vector.tensor_scalar_mul(
                            o_t[:], po[:, lo:hi], r_pt[:, mt:mt + 1]
                        )
                        nc.sync.dma_start(
                            out[mt * P:(mt + 1) * P,
                                ec * 512 + lo:ec * 512 + hi], o_t[:]
                        )

    return nc


_PROGRAM = None


def _get_program() -> bass.Bass:
    global _PROGRAM
    if _PROGRAM is None:
        _PROGRAM = build_program()
    return _PROGRAM


def _prep_inputs(x, enc_q, enc_k, i_mask, Wq, Wk, Wv):
    """Shard + marshal the full fp32 inputs into per-core bf16 DMA layouts."""
    f32 = np.float32
    scale = f32(1.0 / np.sqrt(D))

    def tile4(a2d, inner):  # [R, C] -> [C//inner, P, R//P(outer-of-R? no:], ...
        # a2d[r, c]; chunk layout [cb, p, rt, cc]: value = a2d[rt*P + p, cb*inner + cc]
        rt = a2d.shape[0] // P
        cb = a2d.shape[1] // inner
        return np.ascontiguousarray(
            a2d.reshape(rt, P, cb, inner).transpose(2, 1, 0, 3).astype(NP_BF16)
        )

    # Fold the two projection weights into one matrix on the host:
    # q2 = enc_q @ (Wq^T/sqrt(D)) @ Wk = enc_q @ Wqk  (one D^3 sgemm, ~0.5s)
    Wqk = (np.asarray(Wq, f32).T * scale) @ np.asarray(Wk, f32)  # [e, d]
    wqk1 = tile4(Wqk, P)                                # [db, p, et, 128] from Wqk[e, d]
    wv1 = tile4(np.asarray(Wv, f32).T, 512)             # [ec, p, dt, 512] from WvT[d, e]

    in_maps = []
    for c in range(N_CORES):
        b, m0 = c // 2, (c % 2) * M
        eq = np.asarray(enc_q[b, m0:m0 + M, :], f32)    # [M, D]
        eqT = np.ascontiguousarray(
            eq.T.reshape(DT // 4, 4, P, M).transpose(0, 2, 1, 3).astype(NP_BF16)
        )                                               # [g, p, j, m], dt = 4g+j
        ek1 = tile4(np.asarray(enc_k[b], f32).T, P)     # [sb, p, dt, 128] from ekT[d, s]
        keep = (~np.asarray(i_mask[b, m0:m0 + M, :]))   # [M, S] bool
        keepT = np.ascontiguousarray(
            keep.T.reshape(ST, P, M).astype(NP_BF16)
        )
        x1 = tile4(np.asarray(x[b], f32), P)            # [db, p, st, 128] from x[s, d]
        in_maps.append({
            "eqT": eqT, "wqk1": wqk1, "ek1": ek1,
            "keepT": keepT, "x1": x1, "wv1": wv1,
        })
    return in_maps


def kernel(x, enc_q, enc_k, i_mask, Wq, Wk, Wv):
    nc = _get_program()
    in_maps = _prep_inputs(x, enc_q, enc_k, i_mask, Wq, Wk, Wv)
    res = run_bass_kernel_spmd(nc, in_maps, list(range(N_CORES)))
    out = np.empty((B, S, D), np.float32)
    for c in range(N_CORES):
        b, m0 = c // 2, (c % 2) * M
        out[b, m0:m0 + M, :] = res.results[c]["out"]
    return out



# revision 20
# speedup vs baseline: 1.0131x; 1.0131x over previous
"""Cross-attention kernel for Trainium2, SPMD across 8 NeuronCores.

Problem: out = softmax(mask(enc_q@Wq^T @ (enc_k@Wk^T)^T / sqrt(D))) @ (x@Wv^T)
Shapes: B=4, SQ=SKV=2048, D=2048, fp32 inputs.

Sharding: q-row parallel. Each core owns 1024 q rows (half a batch element).
The chain is reassociated so every stage is row-parallel in q with zero
cross-core communication. The two leading projections are merged by folding
the weight-weight product on the host:
    Wqk  = (Wq^T/sqrt(D)) @ Wk                        (host, one D^3 sgemm)
    q2^T = Wqk-stationary matmul over enc_q^T         [d, m]
    s^T  = enc_k^T-stationary matmul -> scores transposed [s, m]
    u^T  = exp(s^T) * keep^T                          (masked, unnormalized)
    Z    = ones^T @ u^T                               (softmax denominators)
    ax^T = x-stationary matmul over u^T               [d, m]
    out  = (ax^T-stationary @ Wv^T) * (1/Z)           [m, e]
All matmuls run in bf16 with fp32 PSUM accumulation; the final output is
normalized and stored in fp32. The softmax max-subtraction is skipped:
scores are ~N(0,1) after the 1/sqrt(D) scale (|s| <= ||q||*||k|| ~ 45 in the
worst case, far below fp32/bf16 exp overflow), and masked entries are exact
zeros via the multiplicative 0/1 mask.
"""

import os
import sys

import numpy as np

for _p in ("/opt/trn_rl_repo",):
    if _p not in sys.path and os.path.isdir(_p):
        sys.path.append(_p)

import ml_dtypes

import concourse.bass as bass
import concourse.mybir as mybir
import concourse.tile as tile
from concourse.bass_utils import run_bass_kernel_spmd
from concourse.vector_clock import ScopedClock

BF16 = mybir.dt.bfloat16
F32 = mybir.dt.float32
NP_BF16 = ml_dtypes.bfloat16

P = 128          # partitions
D = 2048         # model dim
S = 2048         # kv sequence per batch
M = 1024         # q rows per core
B = 4
N_CORES = 8
DT = D // P      # 16 d-tiles
ST = S // P      # 16 s-tiles
ET = D // P      # 16 e-tiles
MT = M // P      # 8 m-tiles
MC = M // 512    # 2 moving chunks of 512
EC = D // 512    # 4 output e-chunks of 512
EXP = mybir.ActivationFunctionType.Exp


# The walrus build in this image lowers at most ONE sync wait per
# instruction (any class) — more fails codegen with "Too many sync wait
# commands".
_MAX_WAITS = 1


class _SplitDrainTileContext(tile.TileContext):
    """TileContext that legalizes per-instruction semaphore waits for the
    walrus build in this image, which lowers at most ~2 sync waits per
    instruction ("Too many sync wait commands" in codegen otherwise).

    - Engine instructions with >2 waits get a chain of same-engine NOPs
      inserted immediately before them, each carrying <=2 of the waits
      (engine streams are strictly in-order, so this is equivalent).
    - DMA instructions execute on autonomous HWDGE queue procs, where a
      preceding engine NOP would not gate them. Excess waits move to an
      SP NOP chain that increments a fresh semaphore; the DMA keeps one
      original wait plus a wait on that semaphore.
    - The exit drain's all-proc wait set is split one-per-NOP the same way.
    Fix semaphores are cleared in the tail alongside Tile's own."""

    def _legalize_waits(self):
        nc = self.nc
        try:
            from concourse.tile_sem_assignment import DMAInst as _DMAInst
        except Exception:
            _DMAInst = ()
        # One shared forwarding semaphore for all DMA fixes: fix chain k
        # (SP, in-order) increments it to k, and the k-th fixed DMA waits
        # for >= k. SP in-order execution makes the value monotonic and
        # each chain's conditions transitively satisfied.
        fix_sem = None
        fix_val = 0
        n_fix = 0
        for f in nc.m.functions:
            for blk in f.blocks:
                il = blk.instructions
                out_l = []
                for inst in il:
                    si = getattr(inst, "sync_info", None)
                    waits = list(si.on_wait) if (si and si.on_wait) else []
                    if len(waits) <= _MAX_WAITS:
                        out_l.append(inst)
                        continue
                    is_dma = isinstance(inst, _DMAInst) or "DMA" in type(inst).__name__
                    if is_dma:
                        # all waits -> SP NOP chain (1 wait each) -> shared sem
                        if fix_sem is None:
                            fix_sem = nc.alloc_semaphore("waitfix_sem")
                        fix_val += 1
                        for ci, w in enumerate(waits):
                            upd = []
                            if ci == len(waits) - 1:
                                upd = [mybir.SyncUpdate(
                                    sync_type="semaphore", id=fix_sem.num,
                                    ant_name=fix_sem.name, update_mode="sem-inc",
                                    update_value=1,
                                )]
                            out_l.append(mybir.InstNoOp(
                                name=f"I-waitfix{n_fix}-{ci}",
                                sync_info=mybir.SyncInfo(on_wait=[w], on_update=upd),
                                bass_nofuse=True,
                                engine=mybir.EngineType.SP,
                            ))
                        si.on_wait = [mybir.SyncWait(
                            sync_type="semaphore", id=fix_sem.num,
                            ant_name=fix_sem.name,
                            wait_mode="sem-ge-imm", wait_value=fix_val,
                        )]
                    else:
                        # same-engine NOP chain before the instruction,
                        # one wait per NOP (engine streams are in-order)
                        keep = waits[-_MAX_WAITS:]
                        excess = waits[:-_MAX_WAITS]
                        for ci, w in enumerate(excess):
                            out_l.append(mybir.InstNoOp(
                                name=f"I-waitfix{n_fix}-{ci}",
                                sync_info=mybir.SyncInfo(on_wait=[w], on_update=[]),
                                bass_nofuse=True,
                                engine=inst.engine,
                            ))
                        si.on_wait = keep
                    n_fix += 1
                    out_l.append(inst)
                if len(out_l) != len(il):
                    il[:] = out_l
        return fix_sem

    def _drain_and_barrier(self, tick_clock, wait_clock):
        nc = self.nc
        fix_sem = self._legalize_waits()
        fix_sems = [fix_sem] if fix_sem is not None else []
        probe = nc.sync.nop(hint="drain_wait_probe")
        wait_clock.add_sem_waits(
            probe.ins, ScopedClock({None: tick_clock.global_clock})
        )
        waits = list(probe.ins.sync_info.on_wait)
        probe.ins.sync_info.on_wait = waits[:1]
        for w in waits[1:]:
            nop = nc.sync.nop(hint="drain_wait_split")
            if nop.ins.sync_info is None:
                nop.ins.sync_info = mybir.SyncInfo(on_wait=[w], on_update=[])
            else:
                nop.ins.sync_info.on_wait = [w]
        nc.sync.drain()
        nc.all_engine_barrier()
        assert self.sems is not None
        popped = nc._tile_sem_poison_stack.pop()
        assert popped is self._sem_poison
        nc.clear_and_free_semaphores(
            list(self.sems.allocated().values()) + fix_sems
        )
        nc.all_engine_barrier()


def build_program() -> bass.Bass:
    nc = bass.Bass("TRN2", target_bir_lowering=False, debug=False, num_devices=1)

    eqT = nc.dram_tensor("eqT", [DT // 4, P, 4, M], BF16, kind="ExternalInput").ap()
    wqk1 = nc.dram_tensor("wqk1", [DT, P, ET, P], BF16, kind="ExternalInput").ap()
    ek1 = nc.dram_tensor("ek1", [ST, P, DT, P], BF16, kind="ExternalInput").ap()
    keepT = nc.dram_tensor("keepT", [ST, P, M], BF16, kind="ExternalInput").ap()
    x1 = nc.dram_tensor("x1", [DT, P, ST, P], BF16, kind="ExternalInput").ap()
    wv1 = nc.dram_tensor("wv1", [EC, P, DT, 512], BF16, kind="ExternalInput").ap()
    out = nc.dram_tensor("out", [M, D], F32, kind="ExternalOutput").ap()

    with _SplitDrainTileContext(nc) as tc:
        with (
            tc.tile_pool(name="acts", bufs=1) as acts,
            tc.tile_pool(name="wstream", bufs=4) as wstream,
            tc.tile_pool(name="wvstream", bufs=2) as wvstream,
            tc.tile_pool(name="keeps", bufs=2) as keeps,
            tc.tile_pool(name="small", bufs=1) as small,
            tc.tile_pool(name="etmps", bufs=4) as etmps,
            tc.tile_pool(name="outs", bufs=4) as outs,
            tc.tile_pool(name="ps", bufs=6, space="PSUM") as ps,
            tc.tile_pool(name="psz", bufs=2, space="PSUM") as psz,
            tc.tile_pool(name="dram", bufs=1, space="DRAM") as dram,
        ):
            # ---- PE pre-warm: dummy matmuls with no input deps keep the
            # HAM busy-window filled during the input DMA prefill (~7us:
            # HWDGE queues take ~6us to start streaming, then the first
            # et-chunks must land), so the first real matmuls run at
            # K=8/8 (2.4 GHz) with their operands resident. 25 N=512
            # matmuls span ~7.1us (8 cold @427ns + 17 warm @216ns).
            warm_src = small.tile([P, 512], BF16, tag="warmsrc")
            nc.vector.memset(warm_src[:], 0.0)
            warm_ps = ps.tile([P, 512], F32, tag="mm", name="warm")
            for _ in range(25):
                nc.tensor.matmul(
                    warm_ps[:], warm_src[:, :P], warm_src[:],
                    start=True, stop=True,
                )

            # ---- resident activations (two rotating slots per tag) ----
            # DMA trigger instructions serialize on the SP engine (~650ns
            # each), so issue order = availability order. The first stage-A
            # matmul needs wqk chunk 0 + eqT chunk 0: put those first.
            # eqT is split per-dt so matmul et can start as chunk et lands.
            # Prefill: the first real matmul needs only eqT et-chunk 0 plus
            # wqk1[0][:, 0:4, :], so those ship as small lead DMAs (384KB)
            # and the rest streams behind them. eqT otherwise arrives in 4
            # batched DMAs of 4 d-tiles each (fewer ~640ns SP trigger
            # slots -> the wqk stream's triggers issue sooner).
            EQG = 4
            eqT_t = [
                acts.tile([P, EQG, M], BF16, tag="eqt", bufs=DT // EQG,
                          name=f"eqT{g}")
                for g in range(DT // EQG)
            ]
            w0_t = wstream.tile([P, ET, P], BF16, tag="wchunk", name="w0")
            nc.sync.dma_start(eqT_t[0][:, 0:1, :], eqT[0][:, 0:1, :])
            nc.sync.dma_start(w0_t[:, 0:4, :], wqk1[0][:, 0:4, :])
            nc.sync.dma_start(eqT_t[0][:, 1:EQG, :], eqT[0][:, 1:EQG, :])
            nc.sync.dma_start(w0_t[:, 4:ET, :], wqk1[0][:, 4:ET, :])
            for g in range(1, DT // EQG):
                nc.sync.dma_start(eqT_t[g][:], eqT[g])

            # ---- A: q2T[d, m] = (Wqk stationary) over enc_q^T ----
            # Wqk = (Wq^T/sqrt(D)) @ Wk is folded on the host, merging the
            # q- and k-projections into one device stage.
            q2T_sb = acts.tile([P, DT, M], BF16, tag="actA", bufs=1)
            for db in range(DT):
                if db == 0:
                    w_t = w0_t
                else:
                    w_t = wstream.tile([P, ET, P], BF16, tag="wchunk")
                    nc.sync.dma_start(w_t[:], wqk1[db])
                psums = [ps.tile([P, 512], F32, tag="mm", name=f"mm{i}") for i in range(MC)]
                for et in range(ET):
                    for mc in range(MC):
                        nc.tensor.matmul(
                            psums[mc][:],
                            w_t[:, et, :],
                            eqT_t[et // EQG][:, et % EQG, mc * 512:(mc + 1) * 512],
                            start=(et == 0),
                            stop=(et == ET - 1),
                        )
                for mc in range(MC):
                    nc.scalar.copy(q2T_sb[:, db, mc * 512:(mc + 1) * 512], psums[mc][:])

            # ---- S3: uT[s, m] = exp(scoresT) * keepT ----
            uT_sb = acts.tile([P, ST, M], BF16, tag="actB", bufs=1)
            for sb in range(ST):
                w_t = wstream.tile([P, DT, P], BF16, tag="wchunk")
                nc.sync.dma_start(w_t[:], ek1[sb])
                k_t = keeps.tile([P, M], BF16, tag="keep")
                nc.sync.dma_start(k_t[:], keepT[sb])
                psums = [ps.tile([P, 512], F32, tag="mm", name=f"mm{i}") for i in range(MC)]
                for dt in range(DT):
                    for mc in range(MC):
                        nc.tensor.matmul(
                            psums[mc][:],
                            w_t[:, dt, :],
                            q2T_sb[:, dt, mc * 512:(mc + 1) * 512],
                            start=(dt == 0),
                            stop=(dt == DT - 1),
                        )
                for mc in range(MC):
                    e_t = etmps.tile([P, 512], BF16, tag="etmp")
                    nc.scalar.activation(e_t[:], psums[mc][:], EXP)
                    nc.vector.tensor_mul(
                        out=uT_sb[:, sb, mc * 512:(mc + 1) * 512],
                        in0=e_t[:],
                        in1=k_t[:, mc * 512:(mc + 1) * 512],
                    )

            # ---- Z: softmax denominators, then r = 1/Z in [m-partition] form ----
            # 8-wide all-ones stationary (1-col stationary matmuls don't
            # pipeline: +~180ns each). The two mc streams go to different
            # 32-col groups of the PE array (tile_position) so each sb's
            # pair of matmuls runs concurrently, and they alternate PSUM
            # banks so consecutive groups ping-pong like the main stages.
            ones_sb = small.tile([P, 8], BF16, tag="ones")
            nc.vector.memset(ones_sb[:], 1.0)
            r_row = small.tile([1, M], F32, tag="rrow")
            zps = [psz.tile([40, 512], F32, tag="z", name=f"z{i}") for i in range(MC)]
            for sb in range(ST):
                for mc in range(MC):
                    nc.tensor.matmul(
                        zps[mc][32 * mc:32 * mc + 8, :],
                        ones_sb[:],
                        uT_sb[:, sb, mc * 512:(mc + 1) * 512],
                        start=(sb == 0),
                        stop=(sb == ST - 1),
                        tile_position=(0, 32 * mc),
                    )
            for mc in range(MC):
                nc.vector.reciprocal(
                    r_row[:, mc * 512:(mc + 1) * 512], zps[mc][32 * mc:32 * mc + 8, :][0:1, :]
                )
            r_dram = dram.tile([M], F32)
            nc.sync.dma_start(r_dram[None, :], r_row[:])
            r_pt = small.tile([P, MT], F32, tag="rpt")
            nc.sync.dma_start(r_pt[:], r_dram.rearrange("(t p) -> p t", p=P))

            # ---- S5: axT[d, m] = (x stationary) over uT ----
            axT_sb = acts.tile([P, DT, M], BF16, tag="actA", bufs=1)
            for db in range(DT):
                w_t = wstream.tile([P, ST, P], BF16, tag="wchunk")
                nc.sync.dma_start(w_t[:], x1[db])
                psums = [ps.tile([P, 512], F32, tag="mm", name=f"mm{i}") for i in range(MC)]
                for st in range(ST):
                    for mc in range(MC):
                        nc.tensor.matmul(
                            psums[mc][:],
                            w_t[:, st, :],
                            uT_sb[:, st, mc * 512:(mc + 1) * 512],
                            start=(st == 0),
                            stop=(st == ST - 1),
                        )
                for mc in range(MC):
                    nc.scalar.copy(axT_sb[:, db, mc * 512:(mc + 1) * 512], psums[mc][:])

            # ---- S6: out[m, e] = (axT stationary @ Wv^T) * r ----
            # The very last tile runs as two 256-wide halves so its r-mul +
            # store-DMA overlap the second half's matmuls, shortening the
            # exposed tail after the final matmul.
            for ec in range(EC):
                wv_t = wvstream.tile([P, DT, 512], BF16, tag="wvchunk")
                nc.sync.dma_start(wv_t[:], wv1[ec])
                for mt in range(MT):
                    last = (ec == EC - 1) and (mt == MT - 1)
                    po = ps.tile([P, 512], F32, tag="mm", name="mmo")
                    if not last:
                        halves = ((0, 512, po),)
                    else:
                        # separate PSUM banks so half B's matmuls don't
                        # share a bank with the DVE read of half A (the
                        # 6-way mm rotation puts consecutive allocations
                        # in different banks)
                        po2 = ps.tile([P, 512], F32, tag="mm", name="mmo2")
                        halves = ((0, 256, po), (256, 512, po2))
                    for lo, hi, pot in halves:
                        for dt in range(DT):
                            nc.tensor.matmul(
                                pot[:, lo:hi],
                                axT_sb[:, dt, mt * P:(mt + 1) * P],
                                wv_t[:, dt, lo:hi],
                                start=(dt == 0),
                                stop=(dt == DT - 1),
                            )
                        o_t = outs.tile([P, hi - lo], F32,
                                        tag="ostage" if not last else "ostage2")
                        nc.vector.tensor_scalar_mul(
                            o_t[:], pot[:, lo:hi], r_pt[:, mt:mt + 1]
                        )
                        nc.sync.dma_start(
                            out[mt * P:(mt + 1) * P,
                                ec * 512 + lo:ec * 512 + hi], o_t[:]
                        )

    return nc


_PROGRAM = None


def _get_program() -> bass.Bass:
    global _PROGRAM
    if _PROGRAM is None:
        _PROGRAM = build_program()
    return _PROGRAM


def _prep_inputs(x, enc_q, enc_k, i_mask, Wq, Wk, Wv):
    """Shard + marshal the full fp32 inputs into per-core bf16 DMA layouts."""
    f32 = np.float32
    scale = f32(1.0 / np.sqrt(D))

    def tile4(a2d, inner):  # [R, C] -> [C//inner, P, R//P(outer-of-R? no:], ...
        # a2d[r, c]; chunk layout [cb, p, rt, cc]: value = a2d[rt*P + p, cb*inner + cc]
        rt = a2d.shape[0] // P
        cb = a2d.shape[1] // inner
        return np.ascontiguousarray(
            a2d.reshape(rt, P, cb, inner).transpose(2, 1, 0, 3).astype(NP_BF16)
        )

    # Fold the two projection weights into one matrix on the host:
    # q2 = enc_q @ (Wq^T/sqrt(D)) @ Wk = enc_q @ Wqk  (one D^3 sgemm, ~0.5s)
    Wqk = (np.asarray(Wq, f32).T * scale) @ np.asarray(Wk, f32)  # [e, d]
    wqk1 = tile4(Wqk, P)                                # [db, p, et, 128] from Wqk[e, d]
    wv1 = tile4(np.asarray(Wv, f32).T, 512)             # [ec, p, dt, 512] from WvT[d, e]

    in_maps = []
    for c in range(N_CORES):
        b, m0 = c // 2, (c % 2) * M
        eq = np.asarray(enc_q[b, m0:m0 + M, :], f32)    # [M, D]
        eqT = np.ascontiguousarray(
            eq.T.reshape(DT // 4, 4, P, M).transpose(0, 2, 1, 3).astype(NP_BF16)
        )                                               # [g, p, j, m], dt = 4g+j
        ek1 = tile4(np.asarray(enc_k[b], f32).T, P)     # [sb, p, dt, 128] from ekT[d, s]
        keep = (~np.asarray(i_mask[b, m0:m0 + M, :]))   # [M, S] bool
        keepT = np.ascontiguousarray(
            keep.T.reshape(ST, P, M).astype(NP_BF16)
        )
        x1 = tile4(np.asarray(x[b], f32), P)            # [db, p, st, 128] from x[s, d]
        in_maps.append({
            "eqT": eqT, "wqk1": wqk1, "ek1": ek1,
            "keepT": keepT, "x1": x1, "wv1": wv1,
        })
    return in_maps


def kernel(x, enc_q, enc_k, i_mask, Wq, Wk, Wv):
    nc = _get_program()
    in_maps = _prep_inputs(x, enc_q, enc_k, i_mask, Wq, Wk, Wv)
    res = run_bass_kernel_spmd(nc, in_maps, list(range(N_CORES)))
    out = np.empty((B, S, D), np.float32)
    for c in range(N_CORES):
        b, m0 = c // 2, (c % 2) * M
        out[b, m0:m0 + M, :] = res.results[c]["out"]
    return out



# revision 21
# speedup vs baseline: 1.0138x; 1.0007x over previous
"""Cross-attention kernel for Trainium2, SPMD across 8 NeuronCores.

Problem: out = softmax(mask(enc_q@Wq^T @ (enc_k@Wk^T)^T / sqrt(D))) @ (x@Wv^T)
Shapes: B=4, SQ=SKV=2048, D=2048, fp32 inputs.

Sharding: q-row parallel. Each core owns 1024 q rows (half a batch element).
The chain is reassociated so every stage is row-parallel in q with zero
cross-core communication. The two leading projections are merged by folding
the weight-weight product on the host:
    Wqk  = (Wq^T/sqrt(D)) @ Wk                        (host, one D^3 sgemm)
    q2^T = Wqk-stationary matmul over enc_q^T         [d, m]
    s^T  = enc_k^T-stationary matmul -> scores transposed [s, m]
    u^T  = exp(s^T) * keep^T                          (masked, unnormalized)
    Z    = ones^T @ u^T                               (softmax denominators)
    ax^T = x-stationary matmul over u^T               [d, m]
    out  = (ax^T-stationary @ Wv^T) * (1/Z)           [m, e]
All matmuls run in bf16 with fp32 PSUM accumulation; the final output is
normalized and stored in fp32. The softmax max-subtraction is skipped:
scores are ~N(0,1) after the 1/sqrt(D) scale (|s| <= ||q||*||k|| ~ 45 in the
worst case, far below fp32/bf16 exp overflow), and masked entries are exact
zeros via the multiplicative 0/1 mask.
"""

import os
import sys

import numpy as np

for _p in ("/opt/trn_rl_repo",):
    if _p not in sys.path and os.path.isdir(_p):
        sys.path.append(_p)

import ml_dtypes

import concourse.bass as bass
import concourse.mybir as mybir
import concourse.tile as tile
from concourse.bass_utils import run_bass_kernel_spmd
from concourse.vector_clock import ScopedClock

BF16 = mybir.dt.bfloat16
F32 = mybir.dt.float32
NP_BF16 = ml_dtypes.bfloat16

P = 128          # partitions
D = 2048         # model dim
S = 2048         # kv sequence per batch
M = 1024         # q rows per core
B = 4
N_CORES = 8
DT = D // P      # 16 d-tiles
ST = S // P      # 16 s-tiles
ET = D // P      # 16 e-tiles
MT = M // P      # 8 m-tiles
MC = M // 512    # 2 moving chunks of 512
EC = D // 512    # 4 output e-chunks of 512
EXP = mybir.ActivationFunctionType.Exp


# The walrus build in this image lowers at most ONE sync wait per
# instruction (any class) — more fails codegen with "Too many sync wait
# commands".
_MAX_WAITS = 1


class _SplitDrainTileContext(tile.TileContext):
    """TileContext that legalizes per-instruction semaphore waits for the
    walrus build in this image, which lowers at most ~2 sync waits per
    instruction ("Too many sync wait commands" in codegen otherwise).

    - Engine instructions with >2 waits get a chain of same-engine NOPs
      inserted immediately before them, each carrying <=2 of the waits
      (engine streams are strictly in-order, so this is equivalent).
    - DMA instructions execute on autonomous HWDGE queue procs, where a
      preceding engine NOP would not gate them. Excess waits move to an
      SP NOP chain that increments a fresh semaphore; the DMA keeps one
      original wait plus a wait on that semaphore.
    - The exit drain's all-proc wait set is split one-per-NOP the same way.
    Fix semaphores are cleared in the tail alongside Tile's own."""

    def _legalize_waits(self):
        nc = self.nc
        try:
            from concourse.tile_sem_assignment import DMAInst as _DMAInst
        except Exception:
            _DMAInst = ()
        # One shared forwarding semaphore for all DMA fixes: fix chain k
        # (SP, in-order) increments it to k, and the k-th fixed DMA waits
        # for >= k. SP in-order execution makes the value monotonic and
        # each chain's conditions transitively satisfied.
        fix_sem = None
        fix_val = 0
        n_fix = 0
        for f in nc.m.functions:
            for blk in f.blocks:
                il = blk.instructions
                out_l = []
                for inst in il:
                    si = getattr(inst, "sync_info", None)
                    waits = list(si.on_wait) if (si and si.on_wait) else []
                    if len(waits) <= _MAX_WAITS:
                        out_l.append(inst)
                        continue
                    is_dma = isinstance(inst, _DMAInst) or "DMA" in type(inst).__name__
                    if is_dma:
                        # all waits -> SP NOP chain (1 wait each) -> shared sem
                        if fix_sem is None:
                            fix_sem = nc.alloc_semaphore("waitfix_sem")
                        fix_val += 1
                        for ci, w in enumerate(waits):
                            upd = []
                            if ci == len(waits) - 1:
                                upd = [mybir.SyncUpdate(
                                    sync_type="semaphore", id=fix_sem.num,
                                    ant_name=fix_sem.name, update_mode="sem-inc",
                                    update_value=1,
                                )]
                            out_l.append(mybir.InstNoOp(
                                name=f"I-waitfix{n_fix}-{ci}",
                                sync_info=mybir.SyncInfo(on_wait=[w], on_update=upd),
                                bass_nofuse=True,
                                engine=mybir.EngineType.SP,
                            ))
                        si.on_wait = [mybir.SyncWait(
                            sync_type="semaphore", id=fix_sem.num,
                            ant_name=fix_sem.name,
                            wait_mode="sem-ge-imm", wait_value=fix_val,
                        )]
                    else:
                        # same-engine NOP chain before the instruction,
                        # one wait per NOP (engine streams are in-order)
                        keep = waits[-_MAX_WAITS:]
                        excess = waits[:-_MAX_WAITS]
                        for ci, w in enumerate(excess):
                            out_l.append(mybir.InstNoOp(
                                name=f"I-waitfix{n_fix}-{ci}",
                                sync_info=mybir.SyncInfo(on_wait=[w], on_update=[]),
                                bass_nofuse=True,
                                engine=inst.engine,
                            ))
                        si.on_wait = keep
                    n_fix += 1
                    out_l.append(inst)
                if len(out_l) != len(il):
                    il[:] = out_l
        return fix_sem

    def _drain_and_barrier(self, tick_clock, wait_clock):
        nc = self.nc
        fix_sem = self._legalize_waits()
        fix_sems = [fix_sem] if fix_sem is not None else []
        probe = nc.sync.nop(hint="drain_wait_probe")
        wait_clock.add_sem_waits(
            probe.ins, ScopedClock({None: tick_clock.global_clock})
        )
        waits = list(probe.ins.sync_info.on_wait)
        probe.ins.sync_info.on_wait = waits[:1]
        for w in waits[1:]:
            nop = nc.sync.nop(hint="drain_wait_split")
            if nop.ins.sync_info is None:
                nop.ins.sync_info = mybir.SyncInfo(on_wait=[w], on_update=[])
            else:
                nop.ins.sync_info.on_wait = [w]
        nc.sync.drain()
        nc.all_engine_barrier()
        assert self.sems is not None
        popped = nc._tile_sem_poison_stack.pop()
        assert popped is self._sem_poison
        nc.clear_and_free_semaphores(
            list(self.sems.allocated().values()) + fix_sems
        )
        # No trailing all_engine_barrier: nothing in-program follows the
        # clears, and the runtime's end-of-execution quiesce orders them
        # before any re-execution of this NEFF.


def build_program() -> bass.Bass:
    nc = bass.Bass("TRN2", target_bir_lowering=False, debug=False, num_devices=1)

    eqT = nc.dram_tensor("eqT", [DT // 4, P, 4, M], BF16, kind="ExternalInput").ap()
    wqk1 = nc.dram_tensor("wqk1", [DT, P, ET, P], BF16, kind="ExternalInput").ap()
    ek1 = nc.dram_tensor("ek1", [ST, P, DT, P], BF16, kind="ExternalInput").ap()
    keepT = nc.dram_tensor("keepT", [ST, P, M], BF16, kind="ExternalInput").ap()
    x1 = nc.dram_tensor("x1", [DT, P, ST, P], BF16, kind="ExternalInput").ap()
    wv1 = nc.dram_tensor("wv1", [EC, P, DT, 512], BF16, kind="ExternalInput").ap()
    out = nc.dram_tensor("out", [M, D], F32, kind="ExternalOutput").ap()

    with _SplitDrainTileContext(nc) as tc:
        with (
            tc.tile_pool(name="acts", bufs=1) as acts,
            tc.tile_pool(name="wstream", bufs=4) as wstream,
            tc.tile_pool(name="wvstream", bufs=2) as wvstream,
            tc.tile_pool(name="keeps", bufs=2) as keeps,
            tc.tile_pool(name="small", bufs=1) as small,
            tc.tile_pool(name="etmps", bufs=4) as etmps,
            tc.tile_pool(name="outs", bufs=4) as outs,
            tc.tile_pool(name="ps", bufs=6, space="PSUM") as ps,
            tc.tile_pool(name="psz", bufs=2, space="PSUM") as psz,
            tc.tile_pool(name="dram", bufs=1, space="DRAM") as dram,
        ):
            # ---- PE pre-warm: dummy matmuls with no input deps keep the
            # HAM busy-window filled during the input DMA prefill (~7us:
            # HWDGE queues take ~6us to start streaming, then the first
            # et-chunks must land), so the first real matmuls run at
            # K=8/8 (2.4 GHz) with their operands resident. 25 N=512
            # matmuls span ~7.1us (8 cold @427ns + 17 warm @216ns).
            warm_src = small.tile([P, 512], BF16, tag="warmsrc")
            nc.vector.memset(warm_src[:], 0.0)
            warm_ps = ps.tile([P, 512], F32, tag="mm", name="warm")
            for _ in range(25):
                nc.tensor.matmul(
                    warm_ps[:], warm_src[:, :P], warm_src[:],
                    start=True, stop=True,
                )

            # ---- resident activations (two rotating slots per tag) ----
            # DMA trigger instructions serialize on the SP engine (~650ns
            # each), so issue order = availability order. The first stage-A
            # matmul needs wqk chunk 0 + eqT chunk 0: put those first.
            # eqT is split per-dt so matmul et can start as chunk et lands.
            # Prefill: the first real matmul needs only eqT et-chunk 0 plus
            # wqk1[0][:, 0:4, :], so those ship as small lead DMAs (384KB)
            # and the rest streams behind them. eqT otherwise arrives in 4
            # batched DMAs of 4 d-tiles each (fewer ~640ns SP trigger
            # slots -> the wqk stream's triggers issue sooner).
            EQG = 4
            eqT_t = [
                acts.tile([P, EQG, M], BF16, tag="eqt", bufs=DT // EQG,
                          name=f"eqT{g}")
                for g in range(DT // EQG)
            ]
            w0_t = wstream.tile([P, ET, P], BF16, tag="wchunk", name="w0")
            nc.sync.dma_start(eqT_t[0][:, 0:1, :], eqT[0][:, 0:1, :])
            nc.sync.dma_start(w0_t[:, 0:4, :], wqk1[0][:, 0:4, :])
            nc.sync.dma_start(eqT_t[0][:, 1:EQG, :], eqT[0][:, 1:EQG, :])
            nc.sync.dma_start(w0_t[:, 4:ET, :], wqk1[0][:, 4:ET, :])
            for g in range(1, DT // EQG):
                nc.sync.dma_start(eqT_t[g][:], eqT[g])

            # ---- A: q2T[d, m] = (Wqk stationary) over enc_q^T ----
            # Wqk = (Wq^T/sqrt(D)) @ Wk is folded on the host, merging the
            # q- and k-projections into one device stage.
            q2T_sb = acts.tile([P, DT, M], BF16, tag="actA", bufs=1)
            for db in range(DT):
                if db == 0:
                    w_t = w0_t
                else:
                    w_t = wstream.tile([P, ET, P], BF16, tag="wchunk")
                    nc.sync.dma_start(w_t[:], wqk1[db])
                psums = [ps.tile([P, 512], F32, tag="mm", name=f"mm{i}") for i in range(MC)]
                for et in range(ET):
                    for mc in range(MC):
                        nc.tensor.matmul(
                            psums[mc][:],
                            w_t[:, et, :],
                            eqT_t[et // EQG][:, et % EQG, mc * 512:(mc + 1) * 512],
                            start=(et == 0),
                            stop=(et == ET - 1),
                        )
                for mc in range(MC):
                    nc.scalar.copy(q2T_sb[:, db, mc * 512:(mc + 1) * 512], psums[mc][:])

            # ---- S3: uT[s, m] = exp(scoresT) * keepT ----
            uT_sb = acts.tile([P, ST, M], BF16, tag="actB", bufs=1)
            for sb in range(ST):
                w_t = wstream.tile([P, DT, P], BF16, tag="wchunk")
                nc.sync.dma_start(w_t[:], ek1[sb])
                k_t = keeps.tile([P, M], BF16, tag="keep")
                nc.sync.dma_start(k_t[:], keepT[sb])
                psums = [ps.tile([P, 512], F32, tag="mm", name=f"mm{i}") for i in range(MC)]
                for dt in range(DT):
                    for mc in range(MC):
                        nc.tensor.matmul(
                            psums[mc][:],
                            w_t[:, dt, :],
                            q2T_sb[:, dt, mc * 512:(mc + 1) * 512],
                            start=(dt == 0),
                            stop=(dt == DT - 1),
                        )
                for mc in range(MC):
                    e_t = etmps.tile([P, 512], BF16, tag="etmp")
                    nc.scalar.activation(e_t[:], psums[mc][:], EXP)
                    nc.vector.tensor_mul(
                        out=uT_sb[:, sb, mc * 512:(mc + 1) * 512],
                        in0=e_t[:],
                        in1=k_t[:, mc * 512:(mc + 1) * 512],
                    )

            # ---- Z: softmax denominators, then r = 1/Z in [m-partition] form ----
            # 8-wide all-ones stationary (1-col stationary matmuls don't
            # pipeline: +~180ns each). The two mc streams go to different
            # 32-col groups of the PE array (tile_position) so each sb's
            # pair of matmuls runs concurrently, and they alternate PSUM
            # banks so consecutive groups ping-pong like the main stages.
            ones_sb = small.tile([P, 8], BF16, tag="ones")
            nc.vector.memset(ones_sb[:], 1.0)
            r_row = small.tile([1, M], F32, tag="rrow")
            zps = [psz.tile([40, 512], F32, tag="z", name=f"z{i}") for i in range(MC)]
            for sb in range(ST):
                for mc in range(MC):
                    nc.tensor.matmul(
                        zps[mc][32 * mc:32 * mc + 8, :],
                        ones_sb[:],
                        uT_sb[:, sb, mc * 512:(mc + 1) * 512],
                        start=(sb == 0),
                        stop=(sb == ST - 1),
                        tile_position=(0, 32 * mc),
                    )
            for mc in range(MC):
                nc.vector.reciprocal(
                    r_row[:, mc * 512:(mc + 1) * 512], zps[mc][32 * mc:32 * mc + 8, :][0:1, :]
                )
            r_dram = dram.tile([M], F32)
            nc.sync.dma_start(r_dram[None, :], r_row[:])
            r_pt = small.tile([P, MT], F32, tag="rpt")
            nc.sync.dma_start(r_pt[:], r_dram.rearrange("(t p) -> p t", p=P))

            # ---- S5: axT[d, m] = (x stationary) over uT ----
            axT_sb = acts.tile([P, DT, M], BF16, tag="actA", bufs=1)
            for db in range(DT):
                w_t = wstream.tile([P, ST, P], BF16, tag="wchunk")
                nc.sync.dma_start(w_t[:], x1[db])
                psums = [ps.tile([P, 512], F32, tag="mm", name=f"mm{i}") for i in range(MC)]
                for st in range(ST):
                    for mc in range(MC):
                        nc.tensor.matmul(
                            psums[mc][:],
                            w_t[:, st, :],
                            uT_sb[:, st, mc * 512:(mc + 1) * 512],
                            start=(st == 0),
                            stop=(st == ST - 1),
                        )
                for mc in range(MC):
                    nc.scalar.copy(axT_sb[:, db, mc * 512:(mc + 1) * 512], psums[mc][:])

            # ---- S6: out[m, e] = (axT stationary @ Wv^T) * r ----
            # The very last tile runs as two 256-wide halves so its r-mul +
            # store-DMA overlap the second half's matmuls, shortening the
            # exposed tail after the final matmul.
            for ec in range(EC):
                wv_t = wvstream.tile([P, DT, 512], BF16, tag="wvchunk")
                nc.sync.dma_start(wv_t[:], wv1[ec])
                for mt in range(MT):
                    last = (ec == EC - 1) and (mt == MT - 1)
                    po = ps.tile([P, 512], F32, tag="mm", name="mmo")
                    if not last:
                        halves = ((0, 512, po),)
                    else:
                        # separate PSUM banks so half B's matmuls don't
                        # share a bank with the DVE read of half A (the
                        # 6-way mm rotation puts consecutive allocations
                        # in different banks)
                        po2 = ps.tile([P, 512], F32, tag="mm", name="mmo2")
                        halves = ((0, 256, po), (256, 512, po2))
                    for lo, hi, pot in halves:
                        for dt in range(DT):
                            nc.tensor.matmul(
                                pot[:, lo:hi],
                                axT_sb[:, dt, mt * P:(mt + 1) * P],
                                wv_t[:, dt, lo:hi],
                                start=(dt == 0),
                                stop=(dt == DT - 1),
                            )
                        o_t = outs.tile([P, hi - lo], F32,
                                        tag="ostage" if not last else "ostage2")
                        nc.vector.tensor_scalar_mul(
                            o_t[:], pot[:, lo:hi], r_pt[:, mt:mt + 1]
                        )
                        nc.sync.dma_start(
                            out[mt * P:(mt + 1) * P,
                                ec * 512 + lo:ec * 512 + hi], o_t[:]
                        )

    return nc


_PROGRAM = None


def _get_program() -> bass.Bass:
    global _PROGRAM
    if _PROGRAM is None:
        _PROGRAM = build_program()
    return _PROGRAM


def _prep_inputs(x, enc_q, enc_k, i_mask, Wq, Wk, Wv):
    """Shard + marshal the full fp32 inputs into per-core bf16 DMA layouts."""
    f32 = np.float32
    scale = f32(1.0 / np.sqrt(D))

    def tile4(a2d, inner):  # [R, C] -> [C//inner, P, R//P(outer-of-R? no:], ...
        # a2d[r, c]; chunk layout [cb, p, rt, cc]: value = a2d[rt*P + p, cb*inner + cc]
        rt = a2d.shape[0] // P
        cb = a2d.shape[1] // inner
        return np.ascontiguousarray(
            a2d.reshape(rt, P, cb, inner).transpose(2, 1, 0, 3).astype(NP_BF16)
        )

    # Fold the two projection weights into one matrix on the host:
    # q2 = enc_q @ (Wq^T/sqrt(D)) @ Wk = enc_q @ Wqk  (one D^3 sgemm, ~0.5s)
    Wqk = (np.asarray(Wq, f32).T * scale) @ np.asarray(Wk, f32)  # [e, d]
    wqk1 = tile4(Wqk, P)                                # [db, p, et, 128] from Wqk[e, d]
    wv1 = tile4(np.asarray(Wv, f32).T, 512)             # [ec, p, dt, 512] from WvT[d, e]

    in_maps = []
    for c in range(N_CORES):
        b, m0 = c // 2, (c % 2) * M
        eq = np.asarray(enc_q[b, m0:m0 + M, :], f32)    # [M, D]
        eqT = np.ascontiguousarray(
            eq.T.reshape(DT // 4, 4, P, M).transpose(0, 2, 1, 3).astype(NP_BF16)
        )                                               # [g, p, j, m], dt = 4g+j
        ek1 = tile4(np.asarray(enc_k[b], f32).T, P)     # [sb, p, dt, 128] from ekT[d, s]
        keep = (~np.asarray(i_mask[b, m0:m0 + M, :]))   # [M, S] bool
        keepT = np.ascontiguousarray(
            keep.T.reshape(ST, P, M).astype(NP_BF16)
        )
        x1 = tile4(np.asarray(x[b], f32), P)            # [db, p, st, 128] from x[s, d]
        in_maps.append({
            "eqT": eqT, "wqk1": wqk1, "ek1": ek1,
            "keepT": keepT, "x1": x1, "wv1": wv1,
        })
    return in_maps


def kernel(x, enc_q, enc_k, i_mask, Wq, Wk, Wv):
    nc = _get_program()
    in_maps = _prep_inputs(x, enc_q, enc_k, i_mask, Wq, Wk, Wv)
    res = run_bass_kernel_spmd(nc, in_maps, list(range(N_CORES)))
    out = np.empty((B, S, D), np.float32)
    for c in range(N_CORES):
        b, m0 = c // 2, (c % 2) * M
        out[b, m0:m0 + M, :] = res.results[c]["out"]
    return out

